# revision 1
# baseline (speedup 1.0000x reference)
"""Chamfer loss kernel for Trainium2 (8 NeuronCores).

Problem: x, y: [4, 3, 8192] f32.  d2[b,n,m] = ||x[b,:,n] - y[b,:,m]||^2.
out = mean_n(min_m d2) + mean_m(min_n d2)  (scalar f32).

Sharding: core c -> batch c//2, point-half c%2.  Each core runs two
symmetric passes (x-side and y-side row-mins over the full opposing
cloud), so every core's outputs are final mins for a disjoint set of
points and no cross-core reduction is needed.

Device math: one K=15 bf16 matmul per (n-tile, m-block) produces
psum[n,m] = y^2[m] - 2*x.y  (to ~2^-18 relative) via hi/lo split rows:

  k 0..2:   W=-2*xh_d  R=yh_d        k 9..11:  W=1  R=hi(y_d^2)
  k 3..5:   W=-2*xl_d  R=yh_d        k 12..14: W=1  R=lo(y_d^2)
  k 6..8:   W=-2*xh_d  R=yl_d

bf16 products are exact in f32 PSUM; only the xl*yl term (~2^-18) is
dropped.  fp32 matmuls would be ~5x slower on the PE (hi/lo double
pass at half stream rate).

Row-min over m is extracted with a custom fused DVE op
(min(in0,in1) + min-accumulate) that consumes one PSUM tile and one
ScalarE-copied SBUF tile per instruction.  The per-point +x^2[n] and
final means are O(N) host post-processing, as is building the split
rows (host numpy, O(N)).
"""

import sys

if '/opt/trn_rl_repo' not in sys.path:
    sys.path.insert(0, '/opt/trn_rl_repo')

import ml_dtypes
import numpy as np

import concourse.bacc as bacc
import concourse.mybir as mybir
import concourse.tile as tile
from concourse.bass_utils import run_bass_kernel_spmd

# The runtime's trace path imports antenv.axon_hooks, which this image may
# lack.  If BASS_TRACE is set in the environment that import would crash a
# plain kernel() call, so pre-register a no-op stub (a real shim installed
# earlier, e.g. by test.py, is left untouched).
try:
    import antenv.axon_hooks  # noqa: F401
except ImportError:
    import types as _types
    _stub = _types.ModuleType("antenv.axon_hooks")
    _stub.get_axon_ntff_profile_hook = lambda: None
    _stub.set_axon_ntff_profile_hook = lambda h: None
    sys.modules["antenv.axon_hooks"] = _stub

import concourse.dve_ops as dve_ops_mod
from concourse.dve_ops import DveOp
from concourse.dve_spec import (Spec, Src0, Src1, C0, minn, lower, AluOp,
                                _has_src1)
from concourse.dve_uop import DveOpSpec

F32 = mybir.dt.float32
BF16 = mybir.dt.bfloat16
NPBF16 = ml_dtypes.bfloat16
BIG = 3.0e38

B = 4
C = 3
K = 15        # split-K augmented contraction dim
NPTS = 8192   # points per cloud
NSHARD = NPTS // 2  # points handled per core per side
N_CORES = 8


def _ref_min2(in0, in1, c0, c1, c2):
    b = np.minimum(in0.astype(np.float32), in1.astype(np.float32))
    return b, np.minimum(
        np.asarray(c0, np.float32).reshape(-1, 1) if np.ndim(c0) else np.float32(c0),
        b.reshape(b.shape[0], -1).min(axis=-1, keepdims=True))


def register_min2():
    """Custom DVE op: out = min(in0, in1); accum_out = min(s0, min(out)).

    The standard-ISA TENSOR_TENSOR_REDUCE opcode is not supported by the
    runtime here, but custom-DVE ops ship their own uop table with the NEFF.
    This fused op consumes two 512-wide tiles per instruction (one PSUM, one
    SBUF), which is what keeps the DVE at ~0.75 cycles per reduced column."""
    name = "CHAMFER_MIN2_REDUCE"
    if name in dve_ops_mod._SUB_OPCODE_FOR_NAME:
        return next(op for op in dve_ops_mod.OPS if op.name == name)
    spec = Spec(body=minn(Src0, Src1), accum=AluOp.MIN, accum_init=C0,
                reference=_ref_min2)
    row = dve_ops_mod._CUSTOM_DVE_ROW_BASE + len(dve_ops_mod.OPS)
    dve_ops_mod._SUB_OPCODE_FOR_NAME[name] = row
    shas = {}
    for ver in ("v3", "v4"):
        uops = lower(spec, ver=ver)
        shas[ver] = DveOpSpec(name=name, opcode=row, uops=uops,
                              rd1_en=_has_src1(spec)).sha(ver)
    op = DveOp(name, spec, subdim=False, uops_sha=shas)
    dve_ops_mod.OPS.append(op)
    dve_ops_mod.CUSTOM_DVE_SPECS[name] = spec
    return op


MIN2 = register_min2()


def _emit_load(nc, pools, w_dram, r_dram, tag, fast_head=False):
    """Chunked input DMAs so the first matmuls can start early.

    With fast_head, the first weight tile (128 cols) and first rhs block
    (512 cols) are tiny leading DMAs so the PE's first matmul unblocks
    as early as possible."""
    const_pool = pools["const"]
    W = const_pool.tile([K, NSHARD], BF16, tag=f"W_{tag}")
    R = const_pool.tile([K, NPTS], BF16, tag=f"R_{tag}")
    if fast_head:
        # HW-DGE queues only (SW-DGE semaphores land ~3us late), ordered by
        # first consumption.  The sequencer needs ~850ns to issue each DMA,
        # so the early rhs chunks go out on the otherwise-idle Scalar and
        # Vector queues in parallel with the Sync queue.
        nc.sync.dma_start(W[:, 0:128], w_dram[:, 0:128])
        nc.sync.dma_start(R[:, 0:512], r_dram[:, 0:512])
        nc.scalar.dma_start(R[:, 512:2048], r_dram[:, 512:2048])
        nc.scalar.dma_start(R[:, 2048:4096], r_dram[:, 2048:4096])
        nc.sync.dma_start(W[:, 128:NSHARD], w_dram[:, 128:NSHARD])
        for j in range(2, 4):
            s = slice(j * (NPTS // 4), (j + 1) * (NPTS // 4))
            nc.sync.dma_start(R[:, s], r_dram[:, s])
    else:
        for j in range(2):
            s = slice(j * (NSHARD // 2), (j + 1) * (NSHARD // 2))
            nc.sync.dma_start(W[:, s], w_dram[:, s])
        for j in range(4):
            s = slice(j * (NPTS // 4), (j + 1) * (NPTS // 4))
            nc.sync.dma_start(R[:, s], r_dram[:, s])
    return W, R


def _emit_pass(nc, tc, pools, W, R, out_dram, tag):
    """One pass: W [K, NSHARD] bf16 weight rows, R [K, NPTS] bf16 rhs rows,
    out [128, NT] f32 row-mins (partition = point % 128, col = point//128).

    Per (n-tile, pair): 2 matmuls fill two single-bank psum tiles
    [128, 512] (8 banks = 4 pairs in flight); ScalarE copies the second
    to SBUF; the fused MIN2 DVE op consumes the (PSUM, SBUF) pair at 2
    inputs/cycle and min-accumulates the row-min; each tile's result
    streams out as a [128, 1] DMA immediately."""
    NT = NSHARD // 128       # weight tiles
    NP = NPTS // 1024        # pair count (each pair covers 1024 m-columns)

    psum_pool = pools["psum"]
    copy_pool = pools["copy"]
    scratch_pool = pools["scratch"]
    accum_pool = pools["accum"]

    minbuf = pools["const"].tile([128, NT], F32, tag=f"minbuf_{tag}")

    for t in range(NT):
        wslice = W[:, t * 128:(t + 1) * 128]
        # the very last pair of the kernel skips the copy+MIN2 chain and
        # reduces both psum tiles directly, shortening the kernel tail
        last_tile = (tag == "b" and t == NT - 1)
        accum = accum_pool.tile([128, NP + (1 if last_tile else 0)], F32,
                                tag="acc")
        for i in range(NP):
            base = i * 1024
            pa = psum_pool.tile([128, 512], F32, tag="ps")
            nc.tensor.matmul(pa[:], wslice,
                             R[:, base:base + 512], start=True, stop=True)
            pb = psum_pool.tile([128, 512], F32, tag="ps")
            nc.tensor.matmul(pb[:], wslice,
                             R[:, base + 512:base + 1024],
                             start=True, stop=True)
            if last_tile and i == NP - 1:
                nc.vector.tensor_reduce(accum[:, i:i + 1], pa[:],
                                        axis=mybir.AxisListType.X,
                                        op=mybir.AluOpType.min)
                nc.vector.tensor_reduce(accum[:, i + 1:i + 2], pb[:],
                                        axis=mybir.AxisListType.X,
                                        op=mybir.AluOpType.min)
                continue
            cp = copy_pool.tile([128, 512], F32, tag="cp")
            nc.scalar.copy(cp[:], pb[:])
            scr = scratch_pool.tile([128, 512], F32, tag="scr")
            nc.vector._custom_dve(MIN2, out=scr[:], in0=pa[:], in1=cp[:],
                                  s0=BIG, accum_out=accum[:, i:i + 1])
        nc.vector.tensor_reduce(minbuf[:, t:t + 1], accum[:],
                                axis=mybir.AxisListType.X,
                                op=mybir.AluOpType.min)
        # stream each tile's result out as soon as it exists, so the kernel
        # tail only waits on one tiny [128,1] transfer
        nc.sync.dma_start(out_dram[:, t:t + 1], minbuf[:, t:t + 1])


def build_program():
    from contextlib import ExitStack
    nc = bacc.Bacc("TRN2", target_bir_lowering=False, debug=False)
    NT = NSHARD // 128

    wa = nc.dram_tensor("wa", [K, NSHARD], BF16, kind="ExternalInput")
    ra = nc.dram_tensor("ra", [K, NPTS], BF16, kind="ExternalInput")
    wb = nc.dram_tensor("wb", [K, NSHARD], BF16, kind="ExternalInput")
    rb = nc.dram_tensor("rb", [K, NPTS], BF16, kind="ExternalInput")
    minx = nc.dram_tensor("minx", [128, NT], F32, kind="ExternalOutput")
    miny = nc.dram_tensor("miny", [128, NT], F32, kind="ExternalOutput")

    with tile.TileContext(nc) as tc:
        with ExitStack() as ctx:
            pools = {
                "const": ctx.enter_context(tc.tile_pool(name="const", bufs=1)),
                "psum": ctx.enter_context(
                    tc.tile_pool(name="psum", bufs=8, space="PSUM")),
                "copy": ctx.enter_context(tc.tile_pool(name="copy", bufs=4)),
                "scratch": ctx.enter_context(tc.tile_pool(name="scr", bufs=3)),
                "accum": ctx.enter_context(tc.tile_pool(name="acc", bufs=2)),
            }
            # all input loads emitted first: pass-B inputs prefetch during
            # pass A instead of queueing behind pass-A's output DMA
            Wa, Ra = _emit_load(nc, pools, wa, ra, "a", fast_head=True)
            Wb, Rb = _emit_load(nc, pools, wb, rb, "b")
            _emit_pass(nc, tc, pools, Wa, Ra, minx, "a")
            _emit_pass(nc, tc, pools, Wb, Rb, miny, "b")
    nc.compile()
    return nc


_cached_nc = None


def _get_nc():
    global _cached_nc
    if _cached_nc is None:
        _cached_nc = build_program()
    return _cached_nc


def _split_w(shard):
    """shard: [3, n] f32 -> [K, n] bf16 weight rows."""
    n = shard.shape[1]
    xh = shard.astype(NPBF16)
    xl = (shard - xh.astype(np.float32)).astype(NPBF16)
    w = np.empty((K, n), NPBF16)
    w[0:3] = (-2.0 * xh.astype(np.float32)).astype(NPBF16)   # exact scale
    w[3:6] = (-2.0 * xl.astype(np.float32)).astype(NPBF16)
    w[6:9] = w[0:3]
    w[9:15] = NPBF16(1.0)
    return w


def _split_r(full):
    """full: [3, m] f32 -> [K, m] bf16 rhs rows."""
    m = full.shape[1]
    yh = full.astype(NPBF16)
    yl = (full - yh.astype(np.float32)).astype(NPBF16)
    sq = (full.astype(np.float32) ** 2)
    sqh = sq.astype(NPBF16)
    sql = (sq - sqh.astype(np.float32)).astype(NPBF16)
    r = np.empty((K, m), NPBF16)
    r[0:3] = yh
    r[3:6] = yh
    r[6:9] = yl
    r[9:12] = sqh
    r[12:15] = sql
    return r


def run_sharded(x, y, trace=False, **kw):
    """Returns (scalar_out, BassKernelResults)."""
    x = np.ascontiguousarray(x, dtype=np.float32)
    y = np.ascontiguousarray(y, dtype=np.float32)
    nc = _get_nc()
    in_maps = []
    for c in range(N_CORES):
        b, h = c // 2, c % 2
        sl = slice(h * NSHARD, (h + 1) * NSHARD)
        in_maps.append({
            "wa": _split_w(x[b, :, sl]),
            "ra": _split_r(y[b]),
            "wb": _split_w(y[b, :, sl]),
            "rb": _split_r(x[b]),
        })
    res = run_bass_kernel_spmd(nc, in_maps, core_ids=list(range(N_CORES)),
                               trace=trace, **kw)

    # Host epilogue: add ||p||^2 for each sharded point, then mean.
    x2 = np.sum(x.astype(np.float64) ** 2, axis=1)  # [B, NPTS]
    y2 = np.sum(y.astype(np.float64) ** 2, axis=1)  # [B, NPTS]
    sx = 0.0
    sy = 0.0
    for c in range(N_CORES):
        b, h = c // 2, c % 2
        sl = slice(h * NSHARD, (h + 1) * NSHARD)
        vx = res.results[c]["minx"].T.reshape(-1).astype(np.float64)
        vy = res.results[c]["miny"].T.reshape(-1).astype(np.float64)
        sx += np.sum(vx + x2[b, sl])
        sy += np.sum(vy + y2[b, sl])
    out = np.float32(sx / (B * NPTS) + sy / (B * NPTS))
    return out, res


def kernel(x, y):
    out, _ = run_sharded(x, y, trace=False)
    return out



# revision 2
# speedup vs baseline: 1.0500x; 1.0500x over previous
"""Chamfer loss kernel v2 for Trainium2 (8 NeuronCores) — compute-once.

Problem: x, y: [4, 3, 8192] f32.  d2[b,n,m] = ||x[b,:,n] - y[b,:,m]||^2.
out = mean_n(min_m d2) + mean_m(min_n d2)  (scalar f32).

Design:
  * Each core computes ONE [4096 n x 8192 m] block of the full distance
    matrix (core c -> batch c//2, x-half c%2), with the COMPLETE d2 in
    PSUM (x^2 and y^2 rows folded into a K=13 split-bf16 matmul), so the
    same PSUM chunk serves both reduction directions:
      - x-side row-min over m: via the fused op's accumulator port
      - y-side col-min over n: via the fused op's output stream
  * Custom DVE op CHAMFER_COLROW_MIN:
      out   = min(in0, in1)            (elementwise running col-min)
      accum = min(s0, rowmin(in0))     (row-min of in0 alone)
    The stock Spec language only reduces the BODY (min(in0,in1) — polluted
    by colmin), so after lower() the accumulator ALU's src1 is rewired
    from PREV_ALU_OUT (body) to PREV_DELAY_0 (Src0).  A hand-written
    2X_1PORT uop program additionally processes packed bf16 pairs at 2
    elem/lane/cycle (lo=min(S0,S1), hi=min(S0H,S1H), acc folds
    min(S0,S0H)), doubling DVE throughput on the ScalarE-copied path.
  * PSUM drain is split: ~30% of [128,2048] units are consumed directly
    by the DVE (f32 psum, REGULAR mode); the rest are first copied
    psum->bf16 scratch by the otherwise-idle ScalarE, then consumed by
    the DVE in 2X mode.  colmin is bf16 (d2 >= 0, so relative error
    ~2^-9 — harmless for a 2e-2 target).
  * PE runs 32x128 row-tiled matmuls alternating tile_position (0,0) /
    (64,0) (W/R replicated at partition bases 0 and 64).
  * y-side finalization: colmin -> 64 PE transposes -> tensor_reduce
    -> ymin [128, 64].  x-side finalization on host from accbuf.
"""

import sys

if '/opt/trn_rl_repo' not in sys.path:
    sys.path.insert(0, '/opt/trn_rl_repo')

import dataclasses

import ml_dtypes
import numpy as np

import concourse.bacc as bacc
import concourse.mybir as mybir
import concourse.tile as tile
from concourse.bass_utils import run_bass_kernel_spmd

try:
    import antenv.axon_hooks  # noqa: F401
except ImportError:
    import types as _types
    _stub = _types.ModuleType("antenv.axon_hooks")
    _stub.get_axon_ntff_profile_hook = lambda: None
    _stub.set_axon_ntff_profile_hook = lambda h: None
    sys.modules["antenv.axon_hooks"] = _stub

import concourse.dve_ops as dve_ops_mod
from concourse.dve_ops import DveOp, _COMPILE_CACHE
from concourse.dve_spec import (Spec, Src0, Src1, C0, minn, lower, AluOp,
                                _has_src1)
from concourse.dve_uop import (AluInp, DveOpSpec, InpSel, OpConfig, OutPath,
                               OutSel, DelayInp, Trigger, UopConfig,
                               UopDpConfig)

F32 = mybir.dt.float32
BF16 = mybir.dt.bfloat16
NPBF16 = ml_dtypes.bfloat16
BIG = 3.0e38

B = 4
K = 13        # split-K rows: 9 cross terms + y^2 hi/lo + x^2 hi/lo
NPTS = 8192
NSHARD = NPTS // 2   # n rows per core
N_CORES = 8
NT = NSHARD // 128   # 32 n-tiles
CHUNK = 1024         # psum chunk width (2 banks)
NP = 4               # m-pairs of 2048 per n-tile
ACC_W = NT * 8       # accbuf columns (8 slots per n-tile)


def _direct(t, p):
    """True -> the DVE drains this (t, p) unit straight from f32 PSUM."""
    return (t * NP + p) % 10 < 3


def _ref_fused(in0, in1, c0, c1, c2):
    """out = min(in0, in1); accum = min(c0, rowmin(in0))  (reference)."""
    out = np.minimum(in0.astype(np.float32), in1.astype(np.float32))
    rows = in0.astype(np.float32).reshape(in0.shape[0], -1).min(axis=-1,
                                                                keepdims=True)
    c0v = (np.asarray(c0, np.float32).reshape(-1, 1)
           if np.ndim(c0) else np.float32(c0))
    return out, np.minimum(c0v, rows)


def _build_2x_uops():
    """2X_1PORT program: per cycle two packed bf16 pairs per lane.

    Input slots (compacted to delay channels in order):
      d0=SRC_0  d1=SRC_1  d2=SRC_0_HI  d3=SRC_1_HI  d4=CONST_0
    Steady state:
      dp0: lo = min(d0, d1)
      dp1: hi = min(d2, d3);          d5 <- lo
      dp2: t  = min(d0, d2);          d6 <- hi
      dp3: acc = min(acc, t)  [a-chain]
      writes: WR0_LO <- d5 (lo), WR0_HI <- d6 (hi)
    Seed state (1 count): acc <- d4 (C0) at dp3."""
    P = DelayInp.PREV_DELAY
    A = DelayInp.PREV_ALU_OUT
    MIN, BYP = AluOp.MIN, AluOp.BYPASS
    INP = [InpSel.SRC_0, InpSel.SRC_1, InpSel.SRC_0_HI, InpSel.SRC_1_HI,
           InpSel.CONST_0, InpSel.ZERO, InpSel.ZERO, InpSel.ZERO]
    # steady enables only the 4 stream slots -> channels d0..d3; d4/d5
    # are free to carry lo/hi to the write stage.  The seed additionally
    # enables CONST_0 (-> d4 there) for the accumulator seed.
    INP_EN_STEADY = [1, 1, 1, 1, 0, 0, 0, 0]
    INP_EN_SEED = [1, 1, 1, 1, 1, 0, 0, 0]

    def dp(op, s0, s1, delay, a=0):
        return UopDpConfig(op=op, alu_src0=s0, alu_src1=s1, delay=list(delay),
                           alu_out_enable=1, swap_enable=0,
                           alu_out_a_enable=a, alu_out_b_enable=0,
                           delay_enable=[1, 1, 1, 1, 1, 1, 0],
                           idx0_sel=0, idx1_sel=0)

    PASS = [P] * 7

    steady = UopConfig(
        inp=list(INP), inp_enable=list(INP_EN_STEADY),
        out={OutPath.WR0_LO: OutSel.DELAY_4, OutPath.WR0_HI: OutSel.DELAY_5,
             OutPath.WR1_LO: OutSel.ALU_OUT, OutPath.WR1_HI: OutSel.ALU_OUT},
        out_enable={OutPath.WR0_LO: 1, OutPath.WR0_HI: 1,
                    OutPath.WR1_LO: 0, OutPath.WR1_HI: 0},
        out_last_subdim_enable=0, force_two_data_zero=0, force_two_data_one=0,
        require_inp0=1, require_inp1=1, repeat_count=0,
        trigger=(Trigger.SRC_TENSOR_DONE, Trigger.NONE, Trigger.NONE),
        next_uop=(0, 0, 0), inc_parameter_index=0, enable_rev_ops=0,
        match_mask=0, valid_match=0, replace_on_match=0, clear_match=0,
        write_predicate_select=0, write_predicate_enable=0, delay_shift8=0,
        index_increment=0, index_clear=0, accum_enabled=1, v4={},
        datapath_config=[
            dp(MIN, AluInp.PREV_DELAY_0, AluInp.PREV_DELAY_1, PASS),
            dp(MIN, AluInp.PREV_DELAY_2, AluInp.PREV_DELAY_3,
               [P, P, P, P, A, P, P]),
            dp(MIN, AluInp.PREV_DELAY_0, AluInp.PREV_DELAY_2,
               [P, P, P, P, P, A, P]),
            dp(MIN, AluInp.CURR_ALU_OUT, AluInp.PREV_ALU_OUT, PASS, a=1),
            dp(BYP, AluInp.PREV_ALU_OUT, AluInp.PREV_ALU_OUT, PASS, a=1),
            dp(BYP, AluInp.PREV_ALU_OUT, AluInp.PREV_ALU_OUT, PASS, a=1),
            dp(BYP, AluInp.PREV_ALU_OUT, AluInp.PREV_ALU_OUT, PASS, a=1),
            dp(BYP, AluInp.PREV_ALU_OUT, AluInp.PREV_ALU_OUT, PASS, a=1),
        ])

    seed = UopConfig(
        inp=list(INP), inp_enable=list(INP_EN_SEED),
        out={OutPath.WR0_LO: OutSel.ALU_OUT, OutPath.WR0_HI: OutSel.ALU_OUT,
             OutPath.WR1_LO: OutSel.ALU_OUT, OutPath.WR1_HI: OutSel.ALU_OUT},
        out_enable={OutPath.WR0_LO: 0, OutPath.WR0_HI: 0,
                    OutPath.WR1_LO: 0, OutPath.WR1_HI: 0},
        out_last_subdim_enable=0, force_two_data_zero=0, force_two_data_one=0,
        require_inp0=0, require_inp1=0, repeat_count=1,
        trigger=(Trigger.COUNT, Trigger.NONE, Trigger.NONE),
        next_uop=(1, 0, 0), inc_parameter_index=0, enable_rev_ops=0,
        match_mask=0, valid_match=0, replace_on_match=0, clear_match=0,
        write_predicate_select=0, write_predicate_enable=0, delay_shift8=0,
        index_increment=0, index_clear=0, accum_enabled=1, v4={},
        datapath_config=[
            dp(BYP, AluInp.PREV_DELAY_0, AluInp.PREV_DELAY_0, PASS),
            dp(BYP, AluInp.PREV_DELAY_0, AluInp.PREV_DELAY_0, PASS),
            dp(BYP, AluInp.PREV_DELAY_0, AluInp.PREV_DELAY_0, PASS),
            dp(BYP, AluInp.PREV_DELAY_4, AluInp.PREV_DELAY_4, PASS, a=1),
            dp(BYP, AluInp.PREV_ALU_OUT, AluInp.PREV_ALU_OUT, PASS, a=1),
            dp(BYP, AluInp.PREV_ALU_OUT, AluInp.PREV_ALU_OUT, PASS, a=1),
            dp(BYP, AluInp.PREV_ALU_OUT, AluInp.PREV_ALU_OUT, PASS, a=1),
            dp(BYP, AluInp.PREV_ALU_OUT, AluInp.PREV_ALU_OUT, PASS, a=1),
        ])
    return [seed, steady]


def register_fused():
    """Custom DVE op: out = min(in0, in1); accum_out = min(s0, rowmin(in0)).

    REGULAR (1x) program: lower()'s accum-over-body output with the
    accumulator ALU src1 rewired from PREV_ALU_OUT (body) to PREV_DELAY_0
    (Src0).  2X_1PORT program is hand-written (see _build_2x_uops).  The
    finished DveOpSpec is seeded into the compile cache so the NEFF table
    generator and the per-emit path both use it."""
    name = "CHAMFER_COLROW_MIN"
    if name in dve_ops_mod._SUB_OPCODE_FOR_NAME:
        return next(op for op in dve_ops_mod.OPS if op.name == name)
    spec = Spec(body=minn(Src0, Src1), accum=AluOp.MIN, accum_init=C0,
                reference=_ref_fused)
    row = dve_ops_mod._CUSTOM_DVE_ROW_BASE + len(dve_ops_mod.OPS)
    dve_ops_mod._SUB_OPCODE_FOR_NAME[name] = row
    shas = {}
    for ver in ("v3", "v4"):
        uops = lower(spec, ver=ver)
        dpcfg = uops[1].datapath_config[1]
        assert dpcfg.op == AluOp.MIN and dpcfg.alu_src1 == AluInp.PREV_ALU_OUT
        uops[1].datapath_config[1] = dataclasses.replace(
            dpcfg, alu_src1=AluInp.PREV_DELAY_0)
        kw = {}
        if ver == "v3":
            # opcode-entry bits 12-13 hold the highest reachable perf-mode
            # slot (stock TENSOR_TENSOR has 3); bit 12 maps to OpConfig's
            # mask_sel field.  1 = 2X_1PORT reachable.
            kw = dict(uops_2x=_build_2x_uops(), perf_max=1,
                      op=OpConfig(mask_sel=1))
        opspec = DveOpSpec(name=name, opcode=row, uops=uops,
                           rd1_en=_has_src1(spec), **kw)
        opspec.validate(ver)
        shas[ver] = opspec.sha(ver)
        _COMPILE_CACHE[(name, ver)] = opspec
    op = DveOp(name, spec, subdim=False, uops_sha=shas)
    dve_ops_mod.OPS.append(op)
    dve_ops_mod.CUSTOM_DVE_SPECS[name] = spec
    return op


FUSED = register_fused()


def build_program():
    from contextlib import ExitStack
    nc = bacc.Bacc("TRN2", target_bir_lowering=False, debug=False)

    w = nc.dram_tensor("w", [K, NSHARD], BF16, kind="ExternalInput")
    r = nc.dram_tensor("r", [K, NPTS], BF16, kind="ExternalInput")
    ident = nc.dram_tensor("ident", [128, 128], BF16, kind="ExternalInput")
    minx = nc.dram_tensor("minx", [128, ACC_W], F32, kind="ExternalOutput")
    ymin = nc.dram_tensor("ymin", [128, NPTS // 128], F32,
                          kind="ExternalOutput")

    with tile.TileContext(nc) as tc:
        with ExitStack() as ctx:
            const = ctx.enter_context(tc.tile_pool(name="const", bufs=1))
            psum_pool = ctx.enter_context(
                tc.tile_pool(name="psum", bufs=3, space="PSUM"))
            fin_pool = ctx.enter_context(
                tc.tile_pool(name="fin", bufs=2, space="PSUM"))
            scr_pool = ctx.enter_context(tc.tile_pool(name="scr", bufs=3))

            Wt = const.tile([128, NSHARD], BF16, tag="Wt")
            Rt = const.tile([128, NPTS], BF16, tag="Rt")
            colmin = const.tile([128, NPTS], BF16, tag="colmin")
            bigbuf = const.tile([128, 2 * CHUNK], BF16, tag="bigbuf")
            accbuf = const.tile([128, ACC_W], F32, tag="accbuf")
            ymint = const.tile([128, NPTS // 128], F32, tag="ymint")
            identt = const.tile([128, 128], BF16, tag="identt")

            nc.sync.dma_start(Wt[0:K, :], w[:, :])
            nc.sync.dma_start(Wt[64:64 + K, :], w[:, :])
            nc.sync.dma_start(Rt[0:K, :], r[:, :])
            nc.sync.dma_start(Rt[64:64 + K, :], r[:, :])
            nc.sync.dma_start(identt[:, :], ident[:, :])
            nc.vector.memset(bigbuf[:, :], BIG)
            nc.vector.memset(accbuf[:, :], BIG)

            # --- main loop: n-tiles outer, m-pairs (2048 cols) inner.
            # Consecutive matmuls alternate PE row-tile positions (0,0) /
            # (64,0) — W/R are replicated at both partition bases — so the
            # PE can overlap the next matmul's weight load / stream with
            # the current one on the other tile.
            ALT_PER_MM = False   # alternate PE tile per matmul vs per n-tile
            mmctr = 0
            for t in range(NT):
                for p in range(NP):
                    mb = p * 2 * CHUNK
                    pts = []
                    for half in range(2):
                        pt = psum_pool.tile([128, CHUNK], F32, tag="ps")
                        for j in range(CHUNK // 512):
                            bp = 64 * ((mmctr if ALT_PER_MM else t) % 2)
                            mmctr += 1
                            wsl = Wt[bp:bp + K, t * 128:(t + 1) * 128]
                            c0 = mb + half * CHUNK + j * 512
                            nc.tensor.matmul(pt[:, j * 512:(j + 1) * 512],
                                             wsl, Rt[bp:bp + K, c0:c0 + 512],
                                             start=True, stop=True)
                        pts.append(pt)
                    c = t * 8 + p * 2
                    if _direct(t, p):
                        for half in range(2):
                            m0 = mb + half * CHUNK
                            csl = colmin[:, m0:m0 + CHUNK]
                            in1 = (bigbuf[:, 0:CHUNK] if t == 0
                                   else colmin[:, m0:m0 + CHUNK])
                            nc.vector._custom_dve(
                                FUSED, out=csl, in0=pts[half][:, :], in1=in1,
                                s0=BIG, accum_out=accbuf[:, c + half:
                                                         c + half + 1])
                    else:
                        sc = scr_pool.tile([128, 2 * CHUNK], BF16, tag="sc")
                        for half in range(2):
                            nc.scalar.copy(sc[:, half * CHUNK:
                                              (half + 1) * CHUNK],
                                           pts[half][:, :])
                        csl = colmin[:, mb:mb + 2 * CHUNK]
                        in1 = (bigbuf[:, :] if t == 0
                               else colmin[:, mb:mb + 2 * CHUNK])
                        nc.vector._custom_dve(
                            FUSED, out=csl, in0=sc[:, :], in1=in1,
                            s0=BIG, accum_out=accbuf[:, c:c + 1])
            nc.sync.dma_start(minx[:, :], accbuf[:, :])

            # --- y-side finalization: transpose colmin, reduce over n
            for blk in range(NPTS // 512):
                tt = fin_pool.tile([128, 4, 128], BF16, tag="fin")
                for j in range(4):
                    mb = (blk * 4 + j) * 128
                    nc.tensor.transpose(tt[:, j, :],
                                        colmin[:, mb:mb + 128],
                                        identt[:, :])
                nc.vector.tensor_reduce(ymint[:, blk * 4:(blk + 1) * 4],
                                        tt[:, :, :],
                                        axis=mybir.AxisListType.X,
                                        op=mybir.AluOpType.min)
            nc.sync.dma_start(ymin[:, :], ymint[:, :])
    nc.compile()
    return nc


_cached_nc = None


def _get_nc():
    global _cached_nc
    if _cached_nc is None:
        _cached_nc = build_program()
    return _cached_nc


def _build_w(shard):
    """shard: [3, n] f32 -> [K, n] bf16 weight rows (x side)."""
    n = shard.shape[1]
    xh = shard.astype(NPBF16)
    xl = (shard - xh.astype(np.float32)).astype(NPBF16)
    sq = np.sum(shard.astype(np.float64) ** 2, axis=0)  # [n]
    sqh = sq.astype(NPBF16)
    sql = (sq - sqh.astype(np.float64)).astype(NPBF16)
    wm = np.empty((K, n), NPBF16)
    wm[0:3] = (-2.0 * xh.astype(np.float32)).astype(NPBF16)
    wm[3:6] = (-2.0 * xl.astype(np.float32)).astype(NPBF16)
    wm[6:9] = wm[0:3]
    wm[9:11] = NPBF16(1.0)
    wm[11] = sqh
    wm[12] = sql
    return wm


def _build_r(full):
    """full: [3, m] f32 -> [K, m] bf16 rhs rows (y side)."""
    m = full.shape[1]
    yh = full.astype(NPBF16)
    yl = (full - yh.astype(np.float32)).astype(NPBF16)
    sq = np.sum(full.astype(np.float64) ** 2, axis=0)  # [m]
    sqh = sq.astype(NPBF16)
    sql = (sq - sqh.astype(np.float64)).astype(NPBF16)
    rm = np.empty((K, m), NPBF16)
    rm[0:3] = yh
    rm[3:6] = yh
    rm[6:9] = yl
    rm[9] = sqh
    rm[10] = sql
    rm[11:13] = NPBF16(1.0)
    return rm


_IDENT = np.eye(128, dtype=NPBF16)


def run_sharded(x, y, trace=False, **kw):
    """Returns (scalar_out, BassKernelResults)."""
    x = np.ascontiguousarray(x, dtype=np.float32)
    y = np.ascontiguousarray(y, dtype=np.float32)
    nc = _get_nc()
    in_maps = []
    for c in range(N_CORES):
        b, h = c // 2, c % 2
        sl = slice(h * NSHARD, (h + 1) * NSHARD)
        in_maps.append({
            "w": _build_w(x[b, :, sl]),
            "r": _build_r(y[b]),
            "ident": _IDENT,
        })
    res = run_bass_kernel_spmd(nc, in_maps, core_ids=list(range(N_CORES)),
                               trace=trace, **kw)

    sx = 0.0
    ym = {}
    for c in range(N_CORES):
        b, h = c // 2, c % 2
        mx = res.results[c]["minx"].astype(np.float64)      # [128, 256]
        sx += mx.reshape(128, NT, 8).min(axis=2).sum()
        yv = res.results[c]["ymin"].astype(np.float64)      # [128, 64]
        ym[b] = yv if b not in ym else np.minimum(ym[b], yv)
    sy = sum(v.sum() for v in ym.values())
    out = np.float32(sx / (B * NPTS) + sy / (B * NPTS))
    return out, res


def kernel(x, y):
    out, _ = run_sharded(x, y, trace=False)
    return out
